# revision 1
# baseline (speedup 1.0000x reference)
"""KMeansQuantizer Trainium2 kernel.

reference: idx[b,t] = argmin_k( ||c_k||^2 - 2 x[b,t]·c_k )  over K=2048 centroids.

Two-pass design, data-parallel over 8 NeuronCores:
  Pass 1 (float32r matmul, ~13-bit mantissa, 1 cyc/row on PE): computes
    s_k = 2 x·c_k - ||c_k||^2 = -d_k for all rows; argmax_k s == argmin_k d
    (ties -> first index, matching DVE max_index semantics). Outputs the argmax
    index plus the top-2 score values per row.
  Repair pass (exact fp32 matmul, 4 cyc/row): rows whose pass-1 top-2 gap is
    below THRESH are host-gathered and recomputed exactly; indices are
    scattered back. Measured on the reference data: every pass-1 flip has
    gap <= 0.021 (THRESH=0.15 is a 7x margin) and 241 rows are flagged
    (capacity 2048). Result matches a pure fp32 kernel (1 residual
    disagreement vs an fp64 argmin out of 32000 — a gap-9e-5 near-tie that
    any fp32 implementation, including the jax reference itself, resolves
    by accumulation-order luck) at ~1/3 of the pure-fp32 device time:
    pass1 ~350-370us + repair ~60us vs fp32 ~1200us per core-execution.
    Startup: the first 4 x tiles ride the gpsimd SWDGE queue in parallel with
    the 8MB centroid HWDGE stream, so PE transposes/matmuls begin as soon as
    the first transposed-centroid chunks land instead of after the full load.

Per 128-row tile: PE-transpose x (scaled by 2 + rounded to f32r during the
PSUM->SBUF drain on ACT), accumulate 8 e-chunk matmuls per 512-wide k-bank
into PSUM; DVE drains PSUM adding the -||c||^2 bias row (replicated to 128
partitions at setup); DVE max/max_index produce the argmax. idx + top-2
values are staged and written back 4 tiles per DMA (f32-encoded; idx < 2^24
so the u32->f32 convert is exact). Centroids are PE-transposed once into
resident SBUF [e,k] chunks; ||c||^2 via ACT Square+accum_out.
"""
import os
import numpy as np

import concourse.bacc as bacc
import concourse.mybir as mybir
import concourse.tile as tile
from concourse.bass_utils import run_bass_kernel_spmd
from concourse.masks import make_identity

B, T, E, K = 16, 2000, 1024, 2048
N_CORES = 8
N_ROWS = B * T                    # 32000
ROWS_PER_CORE = 4096              # padded total 32768
N_TILES = ROWS_PER_CORE // 128    # 32
EC = E // 128                     # 8 e-chunks
KBANKS = K // 512                 # 4 psum banks of 512
OGROUP = 4                        # row tiles per output DMA

REPAIR_TILES = 1                  # per-core row tiles in the repair pass
REPAIR_CAP = N_CORES * REPAIR_TILES * 128   # 1024 rows
THRESH = 0.15                     # top-2 gap below this -> exact recompute
# measured on the reference data: all f32r flips have gap<=0.021 (7x margin)
# and 241 rows fall under THRESH (4.2x below capacity)

F32 = mybir.dt.float32
F32R = mybir.dt.float32r
U32 = mybir.dt.uint32


def build(mm_dt, n_tiles=N_TILES, reps=1):
    """One NeuronCore program: [n_tiles*128, E] rows -> per row the argmax
    index and top-2 values, packed as f32 triples. reps>1 repeats everything
    (for marginal HW timing)."""
    nc = bacc.Bacc("TRN2", target_bir_lowering=False, debug=False)

    rows = n_tiles * 128
    n_og = (n_tiles + OGROUP - 1) // OGROUP
    x_d = nc.dram_tensor("x", [rows, E], F32, kind="ExternalInput")
    c_d = nc.dram_tensor("c", [K, E], F32, kind="ExternalInput")
    # per row-tile 3 f32 columns: [idx, val0, val1]
    out_d = nc.dram_tensor("out", [n_og, 128, 3 * OGROUP], F32,
                           kind="ExternalOutput")

    with tile.TileContext(nc) as tc:
        with (
            tc.tile_pool(name="const", bufs=1) as constp,
            tc.tile_pool(name="ctp", bufs=1) as ctp,
            tc.tile_pool(name="stage", bufs=2) as stage,
            tc.tile_pool(name="xin", bufs=4) as xin,
            tc.tile_pool(name="xtpool", bufs=2) as xtpool,
            tc.tile_pool(name="dst", bufs=3) as dst,
            tc.tile_pool(name="mxp", bufs=3) as mxp,
            tc.tile_pool(name="og", bufs=2) as ogp,
            tc.tile_pool(name="psum", bufs=4, space="PSUM") as psum,
        ):
            ident = constp.tile([128, 128], F32)
            make_identity(nc, ident)

            for _rep in range(reps):
                # prologue x loads first: they ride the SWDGE queue while the
                # 8MB centroid stream occupies HWDGE, so tile-0/1 transposes
                # are ready as soon as the first cT chunks land
                x_nat = {}
                xT = {}
                ostg = {}

                def load_x(t):
                    if t >= n_tiles:
                        return
                    x_nat[t] = xin.tile([128, E], F32, tag="x_nat", name=f"x{t}")
                    eng = nc.gpsimd if t < 4 else nc.sync
                    eng.dma_start(x_nat[t], x_d[t * 128:(t + 1) * 128, :])

                for _t in range(min(4, n_tiles)):
                    load_x(_t)

                # ---- setup: centroid transpose + norms ----
                cT = []
                for i in range(EC):
                    cti = ctp.tile([128, K], mm_dt, tag=f"ct{i}", name=f"ct{i}")
                    cT.append(cti)
                norms16 = constp.tile([128, 16], F32)
                sq_junk = constp.tile([128, E], F32)
                for j in range(K // 128):          # 16 k-chunks
                    c_nat = stage.tile([128, E], F32, tag="c_nat", name=f"c_nat{j}")
                    # split the startup-critical 8MB codebook stream across
                    # both HWDGE queues (ACT's queue is otherwise idle here)
                    ceng = nc.sync if j % 2 == 0 else nc.scalar
                    ceng.dma_start(c_nat, c_d[j * 128:(j + 1) * 128, :])
                    nc.scalar.activation(
                        sq_junk, c_nat, mybir.ActivationFunctionType.Square,
                        accum_out=norms16[:, j:j + 1],
                    )
                    # 4 transposed e-chunks per psum bank, one ACT drain each
                    for h in range(2):
                        tp = psum.tile([128, 512], F32, tag="xtp",
                                       name=f"ctp{j}_{h}")
                        for q in range(4):
                            i = 4 * h + q
                            nc.tensor.transpose(
                                tp[:, q * 128:(q + 1) * 128],
                                c_nat[:, i * 128:(i + 1) * 128], ident)
                        for q in range(4):
                            i = 4 * h + q
                            nc.scalar.copy(cT[i][:, j * 128:(j + 1) * 128],
                                           tp[:, q * 128:(q + 1) * 128])

                # norms16 [128,16] -> transpose -> negate -> flat bias row,
                # replicated to 128 partitions by doubling DMAs (SWDGE).
                ntp = psum.tile([16, 128], F32, tag="xtp", name="ntp")
                nc.tensor.transpose(ntp, norms16, ident)
                nneg16 = constp.tile([16, 128], F32)
                nc.scalar.mul(nneg16, ntp, -1.0)
                bias128 = constp.tile([128, K], F32)
                # partition-major flatten [16,128] -> [1, 2048] in one DMA
                nc.gpsimd.dma_start(bias128[0:1, :], nneg16[:, :])
                p = 1
                while p < 128:
                    nc.gpsimd.dma_start(bias128[p:2 * p, :], bias128[0:p, :])
                    p *= 2

                # ---- main loop (software-pipelined transposes) ----
                def transpose_half(t, h):
                    """Transpose e-chunks 4h..4h+3 of tile t into one psum
                    bank; ACT drains with scale 2 (+ f32r rounding)."""
                    if t >= n_tiles:
                        return
                    if t not in xT:
                        xT[t] = xtpool.tile([128, E], mm_dt, tag="xT",
                                            name=f"xT{t}")
                    tp = psum.tile([128, 512], F32, tag="xtp", name=f"xtp{t}_{h}")
                    for q in range(4):
                        i = 4 * h + q
                        nc.tensor.transpose(tp[:, q * 128:(q + 1) * 128],
                                            x_nat[t][:, i * 128:(i + 1) * 128],
                                            ident)
                    nc.scalar.mul(xT[t][:, h * 512:(h + 1) * 512], tp, 2.0)

                transpose_half(0, 0)
                transpose_half(0, 1)

                for t in range(n_tiles):
                    if t + 2 >= 4:
                        load_x(t + 2)
                    dist = dst.tile([128, K], F32, tag="dist", name=f"dist{t}")
                    for bp in range(2):           # bank pairs share stationaries
                        pds = [psum.tile([128, 512], F32, tag="pd",
                                         name=f"pd{t}_{2*bp+q}") for q in range(2)]
                        for i in range(EC):
                            for q in range(2):
                                b = 2 * bp + q
                                nc.tensor.matmul(
                                    pds[q],
                                    xT[t][:, i * 128:(i + 1) * 128],
                                    cT[i][:, b * 512:(b + 1) * 512],
                                    start=(i == 0),
                                    stop=(i == EC - 1),
                                )
                        transpose_half(t + 1, bp)
                        for q in range(2):
                            b = 2 * bp + q
                            nc.vector.tensor_add(dist[:, b * 512:(b + 1) * 512],
                                                 pds[q],
                                                 bias128[:, b * 512:(b + 1) * 512])
                    x_nat.pop(t, None)
                    xT.pop(t, None)

                    mx = mxp.tile([128, 8], F32, tag="mx", name=f"mx{t}")
                    mi = mxp.tile([128, 8], U32, tag="mi", name=f"mi{t}")
                    nc.vector.max(out=mx, in_=dist)
                    nc.vector.max_index(out=mi, in_max=mx, in_values=dist)

                    g, r = divmod(t, OGROUP)
                    if r == 0:
                        ostg[g] = ogp.tile([128, 3 * OGROUP], F32, tag="ostg",
                                           name=f"ostg{g}")
                    # u32 idx -> f32 convert is exact (idx < 2^24)
                    nc.vector.tensor_copy(ostg[g][:, 3 * r:3 * r + 1], mi[:, 0:1])
                    nc.vector.tensor_copy(ostg[g][:, 3 * r + 1:3 * r + 3],
                                          mx[:, 0:2])
                    if r == OGROUP - 1 or t == n_tiles - 1:
                        nc.sync.dma_start(out_d[g, :, :], ostg[g])
                        ostg.pop(g, None)

    nc.compile()
    return nc


_cache = {}


def _get_nc(key, **kw):
    if key not in _cache:
        _cache[key] = build(**kw)
    return _cache[key]


def _run_pass(nc, in_maps, n_tiles):
    res = run_bass_kernel_spmd(nc, in_maps, core_ids=list(range(N_CORES)))
    # out: [n_og, 128, 3*OGROUP] f32 per core -> idx/val per row
    idxs, vals = [], []
    for r in res.results:
        o = r["out"]                              # [n_og, 128, 3*OGROUP]
        n_og = o.shape[0]
        o = o.reshape(n_og, 128, OGROUP, 3).transpose(0, 2, 1, 3)  # [g, r, p, 3]
        o = o.reshape(n_og * OGROUP * 128, 3)[:n_tiles * 128]
        idxs.append(o[:, 0].astype(np.int64))
        vals.append(o[:, 1:3].astype(np.float32))
    return np.concatenate(idxs), np.concatenate(vals), res


def run_pass1(x_flat, c, mm_dt=F32R):
    """x_flat [N_ROWS, E] -> idx [32768], val [32768, 2]"""
    xp = np.zeros((ROWS_PER_CORE * N_CORES, E), dtype=np.float32)
    xp[:N_ROWS] = x_flat
    in_maps = [
        {"x": np.ascontiguousarray(xp[i * ROWS_PER_CORE:(i + 1) * ROWS_PER_CORE]),
         "c": c}
        for i in range(N_CORES)
    ]
    nc = _get_nc(("p1", str(mm_dt)), mm_dt=mm_dt)
    return _run_pass(nc, in_maps, N_TILES)


def run_repair(x_rows, c):
    """x_rows [<=REPAIR_CAP, E] -> exact fp32 idx [REPAIR_CAP]"""
    rows_per_core = REPAIR_TILES * 128
    xg = np.zeros((REPAIR_CAP, E), dtype=np.float32)
    xg[:len(x_rows)] = x_rows
    in_maps = [
        {"x": np.ascontiguousarray(xg[i * rows_per_core:(i + 1) * rows_per_core]),
         "c": c}
        for i in range(N_CORES)
    ]
    nc = _get_nc(("rep",), mm_dt=F32, n_tiles=REPAIR_TILES)
    idx, val, res = _run_pass(nc, in_maps, REPAIR_TILES)
    return idx


def kernel(x, centroids):
    x_flat = np.ascontiguousarray(
        np.asarray(x, dtype=np.float32).reshape(N_ROWS, E))
    c = np.ascontiguousarray(np.asarray(centroids, dtype=np.float32))

    mode = os.environ.get("KMEANS_MODE", "f32r+repair")
    if mode == "f32":
        idx, _, _ = run_pass1(x_flat, c, mm_dt=F32)
        return idx[:N_ROWS].reshape(B, T)

    idx, val, _ = run_pass1(x_flat, c, mm_dt=F32R)
    idx = idx[:N_ROWS]
    if mode != "f32r":  # f32r+repair
        gap = (val[:N_ROWS, 0] - val[:N_ROWS, 1])
        suspects = np.flatnonzero(gap < THRESH)
        if len(suspects) > REPAIR_CAP:   # keep the narrowest gaps
            suspects = suspects[np.argsort(gap[suspects])[:REPAIR_CAP]]
        if len(suspects):
            fixed = run_repair(x_flat[suspects], c)
            idx = idx.copy()
            idx[suspects] = fixed[:len(suspects)]
    return idx.reshape(B, T)



# revision 5
# speedup vs baseline: 1.8271x; 1.8271x over previous
"""KMeansQuantizer Trainium2 kernel, v2.

reference: idx[b,t] = argmin_k ||x[b,t] - c_k||^2 over K=2048 centroids
         = argmax_k ( x.c_k - ||c_k||^2/2 )

Data-parallel over 8 NeuronCores (4096 rows each, padded from 32000).

Device program is pure matmul + argmax — all layout work is hoisted to the
host so the PE does nothing but roofline f32r matmuls:
  - x is pre-transposed on the host to [E, rows] per core, so the per-tile
    PE transposes of the baseline (and their PSUM traffic + ACT drains) are
    gone entirely.
  - centroids are pre-transposed to [E, K] on the host; no on-device
    codebook transpose pass (the baseline spent ~25us of PE+ACT+DMA there).
  - the -||c||^2/2 bias row is computed on the host and shipped replicated
    to all 128 partitions ([128, K] input), so no on-device norms/replication.

Per 128-row tile: 8 stationary loads (one per e-chunk), 4 psum banks of
512 k-columns each, 32 f32r matmuls (1 cyc/row, 213ns each) = 6.8us PE.
DVE drains each psum half-bank with a fused tensor_tensor_reduce
(dist = psum + bias, accum_out = running max per half-bank) — 8 x 256-wide
ops — then one 8-wide max8 over the half-bank maxima and a single
max_index scan of the 2048-wide dist row: ~5.1us DVE per tile, safely
under the PE time, so DVE stays off the critical path. PSUM uses all 8
banks (4 per tile, double-buffered across tiles).

Accuracy: f32r matmul error here is ~0.008 absolute on scores whose
typical top-2 gap is ~8, giving ~16 row flips per 32k rows (mean rel err
8.5e-4, 23x inside the 2e-2 gate) with no repair at all. We still repair:
rows whose half-bank top-2 gap is < THRESH are recomputed exactly on the
host (a few hundred rows, ~1 GFLOP of numpy) — final mismatches ~1-5 of
32000 (the residual ones are half-bank-collision near-ties, bounded by
the same f32r error).
"""
import numpy as np

import concourse.bacc as bacc
import concourse.mybir as mybir
import concourse.tile as tile
from concourse.bass_utils import run_bass_kernel_spmd

B, T, E, K = 16, 2000, 1024, 2048
N_CORES = 8
N_ROWS = B * T                    # 32000
ROWS_PER_CORE = 4096              # padded total 32768
N_TILES = ROWS_PER_CORE // 128    # 32
EC = E // 128                     # 8 e-chunks
NBANK = 4                         # psum banks of 512 k-columns
OGROUP = 4                        # row tiles per output DMA
QCOLS = 1024                      # x columns (= rows of x) per streamed chunk
TPQ = QCOLS // 128                # row tiles per chunk
NQ = ROWS_PER_CORE // QCOLS       # 4 streamed chunks

THRESH = 0.075                    # half-bank top-2 gap below this -> host repair
NEG_INF = -3.0e38

F32 = mybir.dt.float32
F32R = mybir.dt.float32r
U32 = mybir.dt.uint32


def build(n_tiles=N_TILES, reps=1, use_ttr=False, cast_dma=True, psum_bufs=8):
    """One NeuronCore program: xT [E, n_tiles*128] (transposed rows),
    cT [E, K], bias [128, K] -> per row argmax index + top-2 half-bank
    maxima, packed as f32 triples."""
    nc = bacc.Bacc("TRN2", target_bir_lowering=False, debug=False)

    rows = n_tiles * 128
    n_og = (n_tiles + OGROUP - 1) // OGROUP
    xT_d = nc.dram_tensor("x", [E, rows], F32, kind="ExternalInput")
    cT_d = nc.dram_tensor("c", [E, K], F32, kind="ExternalInput")
    b_d = nc.dram_tensor("b", [128, K], F32, kind="ExternalInput")
    out_d = nc.dram_tensor("out", [n_og, 128, 3 * OGROUP], F32,
                           kind="ExternalOutput")

    with tile.TileContext(nc) as tc:
        with (
            tc.tile_pool(name="const", bufs=1) as constp,
            tc.tile_pool(name="ctp", bufs=1) as ctp,
            tc.tile_pool(name="cstage", bufs=2) as cstage,
            tc.tile_pool(name="xq", bufs=2) as xqp,
            tc.tile_pool(name="dst", bufs=3) as dst,
            tc.tile_pool(name="mxp", bufs=3) as mxp,
            tc.tile_pool(name="og", bufs=2) as ogp,
            tc.tile_pool(name="psum", bufs=psum_bufs, space="PSUM") as psum,
        ):
            for _rep in range(reps):
                # codebook: 8 e-chunks of [128, K] via both HWDGE queues into
                # f32 stage tiles, ACT-converted to resident f32r tiles
                cT = []
                for i in range(EC):
                    cst = cstage.tile([128, K], F32, tag="cstage",
                                      name=f"cst{i}")
                    ceng = nc.sync if i % 2 == 0 else nc.scalar
                    ceng.dma_start(cst, cT_d[i * 128:(i + 1) * 128, :])
                    cti = ctp.tile([128, K], F32R, tag=f"ct{i}", name=f"ct{i}")
                    nc.scalar.copy(cti, cst)
                    cT.append(cti)

                # bias rides sync behind the codebook (first needed by DVE,
                # well after the first matmuls)
                bias = constp.tile([128, K], F32, tag="bias", name="bias")
                nc.sync.dma_start(bias, b_d[:, :])

                # x streamed in [E, QCOLS] chunks (8 e-chunk tiles each) on
                # the SWDGE queue, double-buffered; the gpsimd DMA casts
                # f32 -> f32r in flight
                xq = {}

                def load_quarter(q):
                    if q >= (n_tiles + TPQ - 1) // TPQ:
                        return
                    tiles = []
                    for i in range(EC):
                        xt = xqp.tile([128, QCOLS], F32R, tag=f"xq{i}",
                                      name=f"xq{q}_{i}")
                        if cast_dma:
                            nc.gpsimd.dma_start(
                                xt, xT_d[i * 128:(i + 1) * 128,
                                         q * QCOLS:(q + 1) * QCOLS])
                        else:
                            xst = cstage.tile([128, QCOLS], F32, tag="xstage",
                                              name=f"xs{q}_{i}")
                            nc.gpsimd.dma_start(
                                xst, xT_d[i * 128:(i + 1) * 128,
                                          q * QCOLS:(q + 1) * QCOLS])
                            nc.scalar.copy(xt, xst)
                        tiles.append(xt)
                    xq[q] = tiles

                load_quarter(0)
                load_quarter(1)

                ostg = {}
                for t in range(n_tiles):
                    q, r = divmod(t, TPQ)
                    if r == 0:
                        load_quarter(q + 2)

                    ps = [psum.tile([128, 512], F32, tag="ps",
                                    name=f"ps{t}_{b}") for b in range(NBANK)]
                    xt = xq[q]
                    for i in range(EC):
                        stat = xt[i][:, r * 128:(r + 1) * 128]
                        for b in range(NBANK):
                            nc.tensor.matmul(
                                ps[b], stat, cT[i][:, b * 512:(b + 1) * 512],
                                start=(i == 0), stop=(i == EC - 1))
                    if r == TPQ - 1:
                        xq.pop(q, None)

                    # fused drain: dist = psum + bias, half-bank running max
                    dist = dst.tile([128, K], F32, tag="dist", name=f"dist{t}")
                    m8 = mxp.tile([128, 8], F32, tag="m8", name=f"m8{t}")
                    mi = mxp.tile([128, 8], U32, tag="mi", name=f"mi{t}")
                    if use_ttr:
                        hmax = mxp.tile([128, 8], F32, tag="hmax",
                                        name=f"hmax{t}")
                        for h in range(8):
                            b, half = divmod(h, 2)
                            nc.vector.tensor_tensor_reduce(
                                out=dist[:, h * 256:(h + 1) * 256],
                                in0=ps[b][:, half * 256:(half + 1) * 256],
                                in1=bias[:, h * 256:(h + 1) * 256],
                                scale=1.0, scalar=NEG_INF,
                                op0=mybir.AluOpType.add,
                                op1=mybir.AluOpType.max,
                                accum_out=hmax[:, h:h + 1])
                        nc.vector.max(out=m8, in_=hmax)
                    else:
                        for b in range(NBANK):
                            nc.vector.tensor_add(
                                dist[:, b * 512:(b + 1) * 512], ps[b],
                                bias[:, b * 512:(b + 1) * 512])
                        nc.vector.max(out=m8, in_=dist)
                    nc.vector.max_index(out=mi, in_max=m8, in_values=dist)

                    g, rr = divmod(t, OGROUP)
                    if rr == 0:
                        ostg[g] = ogp.tile([128, 3 * OGROUP], F32, tag="ostg",
                                           name=f"ostg{g}")
                    # u32 idx -> f32 convert is exact (idx < 2^24);
                    # staging copies ride gpsimd to keep DVE under the PE time
                    nc.gpsimd.tensor_copy(ostg[g][:, 3 * rr:3 * rr + 1],
                                          mi[:, 0:1])
                    nc.gpsimd.tensor_copy(ostg[g][:, 3 * rr + 1:3 * rr + 3],
                                          m8[:, 0:2])
                    if rr == OGROUP - 1 or t == n_tiles - 1:
                        nc.sync.dma_start(out_d[g, :, :], ostg[g])
                        ostg.pop(g, None)

    nc.compile()
    return nc


_cache = {}


def _get_nc(key, **kw):
    if key not in _cache:
        _cache[key] = build(**kw)
    return _cache[key]


def make_in_maps(x_flat_padded, c):
    """x_flat_padded [N_CORES*ROWS_PER_CORE, E] f32, c [K, E] f32 ->
    per-core input dicts with host-transposed layouts."""
    xT = np.ascontiguousarray(
        x_flat_padded.reshape(N_CORES, ROWS_PER_CORE, E).transpose(0, 2, 1))
    cT = np.ascontiguousarray(c.T)
    nrm = np.einsum("ke,ke->k", c.astype(np.float64), c.astype(np.float64))
    bias = np.ascontiguousarray(
        np.broadcast_to((-0.5 * nrm).astype(np.float32)[None, :], (128, K)))
    return [{"x": xT[i], "c": cT, "b": bias} for i in range(N_CORES)], nrm


def _decode(res, n_tiles=N_TILES):
    idxs, vals = [], []
    for r in res.results:
        o = r["out"]                              # [n_og, 128, 3*OGROUP]
        n_og = o.shape[0]
        o = o.reshape(n_og, 128, OGROUP, 3).transpose(0, 2, 1, 3)
        o = o.reshape(n_og * OGROUP * 128, 3)[:n_tiles * 128]
        idxs.append(o[:, 0].astype(np.int64))
        vals.append(o[:, 1:3].astype(np.float32))
    return np.concatenate(idxs), np.concatenate(vals)


def kernel(x, centroids):
    x_flat = np.asarray(x, dtype=np.float32).reshape(N_ROWS, E)
    c = np.ascontiguousarray(np.asarray(centroids, dtype=np.float32))

    xp = np.zeros((N_CORES * ROWS_PER_CORE, E), dtype=np.float32)
    xp[:N_ROWS] = x_flat
    in_maps, nrm = make_in_maps(xp, c)

    nc = _get_nc(("p1",))
    res = run_bass_kernel_spmd(nc, in_maps, core_ids=list(range(N_CORES)))
    idx, val = _decode(res)
    idx = idx[:N_ROWS]

    # host repair: exact fp64 argmin for rows whose half-bank top-2 gap is
    # within the f32r error margin
    gap = val[:N_ROWS, 0] - val[:N_ROWS, 1]
    suspects = np.flatnonzero(gap < THRESH)
    if len(suspects):
        d = nrm[None, :] - 2.0 * (x_flat[suspects].astype(np.float64)
                                  @ c.T.astype(np.float64))
        idx = idx.copy()
        idx[suspects] = np.argmin(d, axis=1)
    return idx.reshape(B, T)


# revision 6
# speedup vs baseline: 3.1475x; 1.7227x over previous
"""KMeansQuantizer Trainium2 kernel, v2.

reference: idx[b,t] = argmin_k ||x[b,t] - c_k||^2 over K=2048 centroids
         = argmax_k ( x.c_k - ||c_k||^2/2 )

Data-parallel over 8 NeuronCores (4096 rows each, padded from 32000).

Device program is pure matmul + argmax — all layout work is hoisted to the
host so the PE does nothing but roofline f32r matmuls:
  - x is pre-transposed on the host to [E, rows] per core, so the per-tile
    PE transposes of the baseline (and their PSUM traffic + ACT drains) are
    gone entirely. The gpsimd (SWDGE) DMA casts f32 -> f32r in flight.
  - centroids are pre-transposed to [E, K] on the host and land via both
    HWDGE queues into f32 stage tiles; one ACT copy per 128-row chunk
    converts to the resident f32r codebook (no on-device transpose pass).
  - the -||c||^2/2 bias row is computed on the host and shipped replicated
    to all 128 partitions ([128, K] input): no on-device norms/replication.

Per 128-row tile: 8 stationary loads (one per e-chunk), 4 psum banks of
512 k-columns each, 32 f32r matmuls (1 cyc/row at ap>=256, 213ns each)
= 6.83us PE at 2.4GHz. DVE drains the 4 banks with tensor_add (+bias),
then max8 over the full 2048-wide dist row (true top-2 values) and a
single max_index scan: ~6.8us DVE, just under the PE time (the two small
output staging copies ride gpsimd to keep it there). PSUM uses all 8
banks (4 per tile, double-buffered across tiles), so matmuls of tile t+1
overlap the DVE drain of tile t.

Measured HW (marginal reps method): 218.4us = 99.9% of the f32r PE
roofline (32 tiles x 16384 moving-column cycles at 2.4GHz = 218.6us);
baseline was 437.6us. NB: tensor_tensor_reduce (fused add+max drain)
hard-crashes the exec unit (NRT_EXEC_UNIT_UNRECOVERABLE) on this
toolchain — the use_ttr=True path is kept for reference but must stay
off.

Accuracy: f32r matmul error is ~0.008 absolute on scores whose typical
top-2 gap is ~8, giving ~16 row flips per 32k rows (mean rel err 8.5e-4,
23x inside the 2e-2 gate) with no repair at all. We still repair: rows
whose top-2 gap is < THRESH are recomputed exactly on the host in fp64
(a few hundred rows, ~1 GFLOP of numpy) — measured 0 mismatches of
32000.
"""
import numpy as np

import concourse.bacc as bacc
import concourse.mybir as mybir
import concourse.tile as tile
from concourse.bass_utils import run_bass_kernel_spmd

B, T, E, K = 16, 2000, 1024, 2048
N_CORES = 8
N_ROWS = B * T                    # 32000
ROWS_PER_CORE = 4096              # padded total 32768
N_TILES = ROWS_PER_CORE // 128    # 32
EC = E // 128                     # 8 e-chunks
NBANK = 4                         # psum banks of 512 k-columns
OGROUP = 4                        # row tiles per output DMA
QCOLS = 1024                      # x columns (= rows of x) per streamed chunk
TPQ = QCOLS // 128                # row tiles per chunk
NQ = ROWS_PER_CORE // QCOLS       # 4 streamed chunks

THRESH = 0.075                    # half-bank top-2 gap below this -> host repair
NEG_INF = -3.0e38

F32 = mybir.dt.float32
F32R = mybir.dt.float32r
U32 = mybir.dt.uint32


def build(n_tiles=N_TILES, reps=1, use_ttr=False, cast_dma=True, psum_bufs=8):
    """One NeuronCore program: xT [E, n_tiles*128] (transposed rows),
    cT [E, K], bias [128, K] -> per row argmax index + top-2 half-bank
    maxima, packed as f32 triples."""
    nc = bacc.Bacc("TRN2", target_bir_lowering=False, debug=False)

    rows = n_tiles * 128
    n_og = (n_tiles + OGROUP - 1) // OGROUP
    xT_d = nc.dram_tensor("x", [E, rows], F32, kind="ExternalInput")
    cT_d = nc.dram_tensor("c", [E, K], F32, kind="ExternalInput")
    b_d = nc.dram_tensor("b", [128, K], F32, kind="ExternalInput")
    out_d = nc.dram_tensor("out", [n_og, 128, 3 * OGROUP], F32,
                           kind="ExternalOutput")

    with tile.TileContext(nc) as tc:
        with (
            tc.tile_pool(name="const", bufs=1) as constp,
            tc.tile_pool(name="ctp", bufs=1) as ctp,
            tc.tile_pool(name="cstage", bufs=2) as cstage,
            tc.tile_pool(name="xq", bufs=2) as xqp,
            tc.tile_pool(name="dst", bufs=3) as dst,
            tc.tile_pool(name="mxp", bufs=3) as mxp,
            tc.tile_pool(name="og", bufs=2) as ogp,
            tc.tile_pool(name="psum", bufs=psum_bufs, space="PSUM") as psum,
        ):
            for _rep in range(reps):
                # codebook: 8 e-chunks of [128, K] via both HWDGE queues into
                # f32 stage tiles, ACT-converted to resident f32r tiles
                cT = []
                for i in range(EC):
                    cst = cstage.tile([128, K], F32, tag="cstage",
                                      name=f"cst{i}")
                    ceng = nc.sync if i % 2 == 0 else nc.scalar
                    ceng.dma_start(cst, cT_d[i * 128:(i + 1) * 128, :])
                    cti = ctp.tile([128, K], F32R, tag=f"ct{i}", name=f"ct{i}")
                    nc.scalar.copy(cti, cst)
                    cT.append(cti)

                # bias rides sync behind the codebook (first needed by DVE,
                # well after the first matmuls)
                bias = constp.tile([128, K], F32, tag="bias", name="bias")
                nc.sync.dma_start(bias, b_d[:, :])

                # x streamed in [E, QCOLS] chunks (8 e-chunk tiles each) on
                # the SWDGE queue, double-buffered; the gpsimd DMA casts
                # f32 -> f32r in flight
                xq = {}

                def load_quarter(q):
                    if q >= (n_tiles + TPQ - 1) // TPQ:
                        return
                    tiles = []
                    for i in range(EC):
                        xt = xqp.tile([128, QCOLS], F32R, tag=f"xq{i}",
                                      name=f"xq{q}_{i}")
                        if cast_dma:
                            nc.gpsimd.dma_start(
                                xt, xT_d[i * 128:(i + 1) * 128,
                                         q * QCOLS:(q + 1) * QCOLS])
                        else:
                            xst = cstage.tile([128, QCOLS], F32, tag="xstage",
                                              name=f"xs{q}_{i}")
                            nc.gpsimd.dma_start(
                                xst, xT_d[i * 128:(i + 1) * 128,
                                          q * QCOLS:(q + 1) * QCOLS])
                            nc.scalar.copy(xt, xst)
                        tiles.append(xt)
                    xq[q] = tiles

                load_quarter(0)
                load_quarter(1)

                ostg = {}
                for t in range(n_tiles):
                    q, r = divmod(t, TPQ)
                    if r == 0:
                        load_quarter(q + 2)

                    ps = [psum.tile([128, 512], F32, tag="ps",
                                    name=f"ps{t}_{b}") for b in range(NBANK)]
                    xt = xq[q]
                    for i in range(EC):
                        stat = xt[i][:, r * 128:(r + 1) * 128]
                        for b in range(NBANK):
                            nc.tensor.matmul(
                                ps[b], stat, cT[i][:, b * 512:(b + 1) * 512],
                                start=(i == 0), stop=(i == EC - 1))
                    if r == TPQ - 1:
                        xq.pop(q, None)

                    # fused drain: dist = psum + bias, half-bank running max
                    dist = dst.tile([128, K], F32, tag="dist", name=f"dist{t}")
                    m8 = mxp.tile([128, 8], F32, tag="m8", name=f"m8{t}")
                    mi = mxp.tile([128, 8], U32, tag="mi", name=f"mi{t}")
                    if use_ttr:
                        hmax = mxp.tile([128, 8], F32, tag="hmax",
                                        name=f"hmax{t}")
                        for h in range(8):
                            b, half = divmod(h, 2)
                            nc.vector.tensor_tensor_reduce(
                                out=dist[:, h * 256:(h + 1) * 256],
                                in0=ps[b][:, half * 256:(half + 1) * 256],
                                in1=bias[:, h * 256:(h + 1) * 256],
                                scale=1.0, scalar=NEG_INF,
                                op0=mybir.AluOpType.add,
                                op1=mybir.AluOpType.max,
                                accum_out=hmax[:, h:h + 1])
                        nc.vector.max(out=m8, in_=hmax)
                    else:
                        for b in range(NBANK):
                            nc.vector.tensor_add(
                                dist[:, b * 512:(b + 1) * 512], ps[b],
                                bias[:, b * 512:(b + 1) * 512])
                        nc.vector.max(out=m8, in_=dist)
                    nc.vector.max_index(out=mi, in_max=m8, in_values=dist)

                    g, rr = divmod(t, OGROUP)
                    if rr == 0:
                        ostg[g] = ogp.tile([128, 3 * OGROUP], F32, tag="ostg",
                                           name=f"ostg{g}")
                    # u32 idx -> f32 convert is exact (idx < 2^24);
                    # staging copies ride gpsimd to keep DVE under the PE time
                    nc.gpsimd.tensor_copy(ostg[g][:, 3 * rr:3 * rr + 1],
                                          mi[:, 0:1])
                    nc.gpsimd.tensor_copy(ostg[g][:, 3 * rr + 1:3 * rr + 3],
                                          m8[:, 0:2])
                    if rr == OGROUP - 1 or t == n_tiles - 1:
                        nc.sync.dma_start(out_d[g, :, :], ostg[g])
                        ostg.pop(g, None)

    nc.compile()
    return nc


_cache = {}


def _get_nc(key, **kw):
    if key not in _cache:
        _cache[key] = build(**kw)
    return _cache[key]


def make_in_maps(x_flat_padded, c):
    """x_flat_padded [N_CORES*ROWS_PER_CORE, E] f32, c [K, E] f32 ->
    per-core input dicts with host-transposed layouts."""
    xT = np.ascontiguousarray(
        x_flat_padded.reshape(N_CORES, ROWS_PER_CORE, E).transpose(0, 2, 1))
    cT = np.ascontiguousarray(c.T)
    nrm = np.einsum("ke,ke->k", c.astype(np.float64), c.astype(np.float64))
    bias = np.ascontiguousarray(
        np.broadcast_to((-0.5 * nrm).astype(np.float32)[None, :], (128, K)))
    return [{"x": xT[i], "c": cT, "b": bias} for i in range(N_CORES)], nrm


def _decode(res, n_tiles=N_TILES):
    idxs, vals = [], []
    for r in res.results:
        o = r["out"]                              # [n_og, 128, 3*OGROUP]
        n_og = o.shape[0]
        o = o.reshape(n_og, 128, OGROUP, 3).transpose(0, 2, 1, 3)
        o = o.reshape(n_og * OGROUP * 128, 3)[:n_tiles * 128]
        idxs.append(o[:, 0].astype(np.int64))
        vals.append(o[:, 1:3].astype(np.float32))
    return np.concatenate(idxs), np.concatenate(vals)


def kernel(x, centroids):
    x_flat = np.asarray(x, dtype=np.float32).reshape(N_ROWS, E)
    c = np.ascontiguousarray(np.asarray(centroids, dtype=np.float32))

    xp = np.zeros((N_CORES * ROWS_PER_CORE, E), dtype=np.float32)
    xp[:N_ROWS] = x_flat
    in_maps, nrm = make_in_maps(xp, c)

    nc = _get_nc(("p1",))
    res = run_bass_kernel_spmd(nc, in_maps, core_ids=list(range(N_CORES)))
    idx, val = _decode(res)
    idx = idx[:N_ROWS]

    # host repair: exact fp64 argmin for rows whose half-bank top-2 gap is
    # within the f32r error margin
    gap = val[:N_ROWS, 0] - val[:N_ROWS, 1]
    suspects = np.flatnonzero(gap < THRESH)
    if len(suspects):
        d = nrm[None, :] - 2.0 * (x_flat[suspects].astype(np.float64)
                                  @ c.T.astype(np.float64))
        idx = idx.copy()
        idx[suspects] = np.argmin(d, axis=1)
    return idx.reshape(B, T)
